# revision 3
# baseline (speedup 1.0000x reference)
"""GNN message-passing (NORMADJ graph conv) on 8 Trainium2 NeuronCores. v3.

Math (reference):
    d_e = pow(diags, e)
    gso_1[e]  = m2 * d_e2[row[e]] * d_e3[col[e]]        edge weights
    gso_2[i]  = m1*d_e1[i] + m2*d_e2[i]*d_e3[i] + m3    self-loop weights
    out[i]    = sum_{e: col[e]==i} gso_1[e] * x[row[e]] + gso_2[i] * x[i]

Key identity: d_e3[col] depends only on the destination, so
    out[i] = m2*d_e3[i] * sum_{e: col[e]==i} (d_e2[row[e]] * x[row[e]]) + gso_2[i]*x[i]
i.e. pre-scale x rows once on device (xs = d_e2 * x), scatter-add gathered xs
rows, post-scale per destination node.

Distribution: edges sharded by DESTINATION node range (8 ranges of 12544
nodes); no collectives. Host does index packing only.

v3: destination tiles are 64 nodes wide (DW=64). The scatter one-hot per
chunk is [128 slots, 64 dests] -> the DVE is_equal work halves vs 128-wide
tiles (59M -> 32M elems) at the cost of ~8% more gather padding. PSUM
accumulators are [64, 64]; the self-loop is folded into the accumulation as
an identity matmul over 64 partitions.
"""

import numpy as np
import ml_dtypes

P = 128                       # slots per chunk (PE contraction width)
D = 64
DW = 64                       # destination nodes per tile
N_CORES = 8
N_NODES = 100000
NPC = 12544                   # nodes per core
TPD = NPC // DW               # destination tiles per core (196)
NPAD = N_CORES * NPC          # padded node count (100352)
NT = NPAD // P                # 128-node tiles for the xs phase (784)
NTD = NPAD // DW              # 64-node dest tiles overall (1568)
NB = 4                        # source banks (int16 gather index limit)
BANK = NPAD // NB             # 25088 rows per bank
TPB = NT // NB                # 128-node tiles per bank (196)
T_GROUP = 8                   # dest tiles per dma_gather instruction
XS_G = 14                     # 128-node tiles per xs pre-scale group
GPB = TPB // XS_G             # xs groups per bank (14)
CAPS_DEFAULT = (5, 5, 5, 5)   # chunks per (tile, bank), fitted to the data

_cache = {}


def _groups():
    gs = []
    t = 0
    while t < TPD:
        gs.append((t, min(T_GROUP, TPD - t)))
        t += min(T_GROUP, TPD - t)
    return gs


def _build_program(caps, n_cores, ablate=()):
    import concourse.bacc as bacc
    import concourse.mybir as mybir
    from concourse.tile import TileContext

    f32 = mybir.dt.float32
    bf16 = mybir.dt.bfloat16
    i16 = mybir.dt.int16
    ACT = mybir.ActivationFunctionType

    KT = sum(caps)
    groups = _groups()
    idx_cols_total = sum(tg * cb * 8 for (_, tg) in groups for cb in caps)

    nc = bacc.Bacc(
        "TRN2", target_bir_lowering=False, debug=False, num_devices=n_cores
    )

    xfull = nc.dram_tensor("xfull", [NPAD, D], f32, kind="ExternalInput")
    diagsT = nc.dram_tensor("diagsT", [P, NT], f32, kind="ExternalInput")
    ddestT = nc.dram_tensor("ddestT", [DW, TPD], f32, kind="ExternalInput")
    xdest = nc.dram_tensor("xdest", [NPC, D], bf16, kind="ExternalInput")
    idx16 = nc.dram_tensor("idx16", [P, idx_cols_total], i16, kind="ExternalInput")
    colrelT = nc.dram_tensor("colrelT", [P, TPD * KT], bf16, kind="ExternalInput")
    iotaq = nc.dram_tensor("iotaq", [P, KT * DW], bf16, kind="ExternalInput")
    identT = nc.dram_tensor("identT", [DW, DW], bf16, kind="ExternalInput")
    scal_in = {
        s: nc.dram_tensor(s + "c", [DW, 1], f32, kind="ExternalInput")
        for s in ("m1", "m2", "m3", "e1", "e2", "e3")
    }
    scal128 = nc.dram_tensor("e2full", [P, 1], f32, kind="ExternalInput")
    out_d = nc.dram_tensor("out", [NPC, D], f32, kind="ExternalOutput")

    with TileContext(nc) as tc:
        with (
            tc.tile_pool(name="const", bufs=1) as const,
            tc.tile_pool(name="vecs", bufs=1) as vecs,
            tc.tile_pool(name="xsload", bufs=6) as xsload,
            tc.tile_pool(name="xsst", bufs=3) as xsst,
            tc.tile_pool(name="idxp", bufs=5) as idxp,
            tc.tile_pool(name="msgb", bufs=8) as msgbp,
            tc.tile_pool(name="stp", bufs=12) as stp,
            tc.tile_pool(name="outp", bufs=6) as outp,
            tc.tile_pool(name="psum", bufs=8, space="PSUM") as psum,
            tc.tile_pool(name="dram", bufs=1, space="DRAM") as dram,
        ):
            # per-node degree vectors first: the xs-phase critical chain
            # (Ln -> Exp -> muls) hangs off these
            diagsT_sb = vecs.tile([P, NT], f32)
            nc.sync.dma_start(out=diagsT_sb[:], in_=diagsT[:])
            ddestT_sb = vecs.tile([DW, TPD], f32)
            nc.sync.dma_start(out=ddestT_sb[:], in_=ddestT[:])

            # resident constants / index tables
            iota_sb = const.tile([P, KT * DW], bf16)
            nc.sync.dma_start(out=iota_sb[:], in_=iotaq[:])
            colT_sb = const.tile([P, TPD * KT], bf16)
            nc.sync.dma_start(out=colT_sb[:], in_=colrelT[:])
            ident_sb = const.tile([DW, DW], bf16)
            nc.sync.dma_start(out=ident_sb[:], in_=identT[:])
            scal = {}
            for s, t in scal_in.items():
                scal[s] = const.tile([DW, 1], f32, name="scal_" + s)
                nc.sync.dma_start(out=scal[s][:], in_=t[:])
            e2f = const.tile([P, 1], f32, name="e2f")
            nc.sync.dma_start(out=e2f[:], in_=scal128[:])

            # ---- per-node scalar vectors ----------------------------------
            ln_all = vecs.tile([P, NT], f32)
            nc.scalar.activation(out=ln_all[:], in_=diagsT_sb[:], func=ACT.Ln)
            ln_dest = vecs.tile([DW, TPD], f32)
            nc.scalar.activation(out=ln_dest[:], in_=ddestT_sb[:], func=ACT.Ln)

            d2_all = vecs.tile([P, NT], f32)
            nc.scalar.activation(
                out=d2_all[:], in_=ln_all[:], func=ACT.Exp, scale=e2f[:, 0:1]
            )
            d1d = vecs.tile([DW, TPD], f32)
            nc.scalar.activation(
                out=d1d[:], in_=ln_dest[:], func=ACT.Exp, scale=scal["e1"][:, 0:1]
            )
            d2d = vecs.tile([DW, TPD], f32)
            nc.scalar.activation(
                out=d2d[:], in_=ln_dest[:], func=ACT.Exp, scale=scal["e2"][:, 0:1]
            )
            d3d = vecs.tile([DW, TPD], f32)
            nc.scalar.activation(
                out=d3d[:], in_=ln_dest[:], func=ACT.Exp, scale=scal["e3"][:, 0:1]
            )
            post3 = vecs.tile([DW, TPD], f32)
            nc.vector.tensor_scalar_mul(
                out=post3[:], in0=d3d[:], scalar1=scal["m2"][:, 0:1]
            )
            gso2 = vecs.tile([DW, TPD], f32)
            nc.vector.tensor_mul(out=gso2[:], in0=d2d[:], in1=post3[:])
            t1 = vecs.tile([DW, TPD], f32)
            nc.vector.tensor_scalar_mul(
                out=t1[:], in0=d1d[:], scalar1=scal["m1"][:, 0:1]
            )
            nc.vector.tensor_add(out=gso2[:], in0=gso2[:], in1=t1[:])
            nc.vector.tensor_scalar_add(
                out=gso2[:], in0=gso2[:], scalar1=scal["m3"][:, 0:1]
            )
            # selfw = gso2 / post3: the self-loop rides the shared post3
            # post-scale (requires post3 != 0, i.e. m2 != 0 as in the spec)
            rp3 = vecs.tile([DW, TPD], f32)
            nc.vector.reciprocal(out=rp3[:], in_=post3[:])
            selfw = vecs.tile([DW, TPD], f32)
            nc.vector.tensor_mul(out=selfw[:], in0=gso2[:], in1=rp3[:])

            # prefetch the first gather groups' index tables so the first
            # gathers do not queue behind the xs writes on SP
            prefetched = {}
            _off = 0
            for gi, (t0, tg) in enumerate(groups[:3]):
                gcols = sum(tg * cb * 8 for cb in caps)
                pidx = idxp.tile([P, gcols], i16, name="idx_sb")
                nc.sync.dma_start(out=pidx[:], in_=idx16[:, _off : _off + gcols])
                prefetched[gi] = pidx
                _off += gcols

            # ---- xs = d2[j] * x[j], bf16 packed into the low half of 256B
            # rows (declared f32 so the gather runs in cheap 4-byte mode).
            # one DRAM tensor per bank so bank-b gathers only wait on bank b.
            # within a group, partition p holds XS_G consecutive nodes so the
            # load is one contiguous 3.5KB descriptor per partition.
            xs_banks = [
                dram.tile([BANK, D], f32, name=f"xsb{b}") for b in range(NB)
            ]
            xf_b = xfull[:].rearrange(
                "(b g p k) d -> b g p k d", b=NB, g=GPB, p=P
            )
            for b in range(NB if "xs" not in ablate else 0):
                xs_v = xs_banks[b][:].rearrange(
                    "(g p k) e -> g p k e", g=GPB, p=P
                )
                for g in range(GPB):
                    xg = xsload.tile([P, XS_G * D], f32, tag="xg")
                    nc.scalar.dma_start(
                        out=xg[:].rearrange("p (k d) -> p k d", d=D),
                        in_=xf_b[b, g],
                    )
                    xsg = xsst.tile([P, XS_G * D], bf16, tag="xsg")
                    c0 = (b * GPB + g) * XS_G
                    nc.vector.tensor_mul(
                        out=xsg[:].rearrange("p (k d) -> p k d", d=D),
                        in0=xg[:].rearrange("p (k d) -> p k d", d=D),
                        in1=d2_all[:, c0 : c0 + XS_G].to_broadcast([P, XS_G, D]),
                    )
                    nc.sync.dma_start(
                        out=xs_v[g][:, :, 0 : D // 2].bitcast(bf16),
                        in_=xsg[:].rearrange("p (k d) -> p k d", d=D),
                    )

            # ---- main loop: gather / select / matmul-accumulate ------------
            iota3 = iota_sb[:].rearrange("p (q c) -> p q c", c=KT)
            idx_off = 0
            for gi, (t0, tg) in enumerate(groups):
                gcols = sum(tg * cb * 8 for cb in caps)
                if gi in prefetched:
                    idx_sb = prefetched[gi]
                else:
                    idx_sb = idxp.tile([P, gcols], i16, name="idx_sb")
                    nc.sync.dma_start(
                        out=idx_sb[:], in_=idx16[:, idx_off : idx_off + gcols]
                    )
                idx_off += gcols
                msgs = []
                boff = 0
                for b in range(NB):
                    cb = caps[b]
                    nidx = tg * cb * P
                    msgb = msgbp.tile([P, tg * cb * D], f32, tag="msgb")
                    nc.gpsimd.dma_gather(
                        out_ap=msgb[:].rearrange("p (k e) -> p k e", e=D),
                        in_ap=xs_banks[b][:],
                        idxs_ap=idx_sb[:, boff : boff + nidx // 16],
                        num_idxs=nidx,
                        num_idxs_reg=nidx,
                        elem_size=D,
                        single_packet=False,
                    )
                    boff += nidx // 16
                    msgs.append(msgb)

                xdest_g = outp.tile([DW, tg * D], bf16, name="xdest_g")
                nc.sync.dma_start(
                    out=xdest_g[:].rearrange("p (t d) -> p t d", d=D),
                    in_=xdest[t0 * DW : (t0 + tg) * DW, :].rearrange(
                        "(t p) d -> p t d", p=DW
                    ),
                )
                out_g = outp.tile([DW, tg * D], f32, name="out_g")
                for q in range(tg):
                    t = t0 + q
                    sT = (
                        stp.tile([P, KT * DW], bf16, name="sT")
                        if "st" not in ablate
                        else iota_sb
                    )
                    sT3 = sT[:].rearrange("p (q c) -> p q c", c=KT)
                    if "st" not in ablate:
                        nc.vector.tensor_tensor(
                            out=sT3,
                            in0=colT_sb[:, t * KT : (t + 1) * KT]
                            .rearrange("p k -> p () k")
                            .to_broadcast([P, DW, KT]),
                            in1=iota3,
                            op=mybir.AluOpType.is_equal,
                        )
                    # self-loop term, scaled so the shared post3 post-scale
                    # yields gso2 * x[i]
                    self_sb = outp.tile([DW, D], bf16, name="self_sb")
                    nc.vector.tensor_scalar_mul(
                        out=self_sb[:],
                        in0=xdest_g[:, q * D : (q + 1) * D],
                        scalar1=selfw[:, t : t + 1],
                    )
                    acc = psum.tile([DW, D], f32, name="acc")
                    if "mm" in ablate:
                        nc.tensor.matmul(
                            out=acc[:],
                            lhsT=sT3[:, :, 0:1],
                            rhs=msgs[0][
                                :, q * caps[0] * D : q * caps[0] * D + D // 2
                            ].bitcast(bf16),
                            start=True,
                            stop=True,
                        )
                    else:
                        j = 0
                        for b in range(NB):
                            cb = caps[b]
                            for k in range(cb):
                                c0 = (q * cb + k) * D
                                nc.tensor.matmul(
                                    out=acc[:],
                                    lhsT=sT3[:, :, j : j + 1],
                                    rhs=msgs[b][:, c0 : c0 + D // 2].bitcast(bf16),
                                    start=(j == 0),
                                    stop=False,
                                )
                                j += 1
                        nc.tensor.matmul(
                            out=acc[:],
                            lhsT=ident_sb[:],
                            rhs=self_sb[:],
                            start=False,
                            stop=True,
                        )
                    nc.scalar.activation(
                        out=out_g[:, q * D : (q + 1) * D], in_=acc[:],
                        func=ACT.Copy, scale=post3[:, t : t + 1],
                    )
                nc.sync.dma_start(
                    out=out_d[t0 * DW : (t0 + tg) * DW, :].rearrange(
                        "(t p) d -> p t d", p=DW
                    ),
                    in_=out_g[:].rearrange("p (t d) -> p t d", d=D),
                )

    nc.compile()
    return nc


def _get_program(caps, n_cores, ablate=()):
    key = (tuple(caps), n_cores, tuple(ablate))
    if key not in _cache:
        _cache[key] = _build_program(tuple(caps), n_cores, ablate)
    return _cache[key]


def compute_caps(col, row):
    tile = col // DW
    bank = row // BANK
    cnt = np.zeros((NTD, NB), np.int64)
    np.add.at(cnt, (tile, bank), 1)
    caps = np.maximum(np.ceil(cnt.max(axis=0) / P).astype(int), 1)
    return tuple(int(max(c, d)) for c, d in zip(caps, CAPS_DEFAULT))


def pack_inputs(x, edge_index, diags, m1, m2, m3, e1, e2, e3, caps):
    """Host-side index packing. Returns list of per-core input dicts."""
    bf16 = ml_dtypes.bfloat16
    row = np.ascontiguousarray(edge_index[0]).astype(np.int64, copy=False)
    col = np.ascontiguousarray(edge_index[1]).astype(np.int64, copy=False)
    KT = sum(caps)
    bank_col_off = np.concatenate([[0], np.cumsum(caps)])  # chunk offsets

    tile = col // DW
    bank = row // BANK
    key = tile * NB + bank
    order = np.argsort(key, kind="stable")
    row_s = row[order]
    col_s = col[order]
    key_s = key[order]

    counts = np.bincount(key_s, minlength=NTD * NB)
    cnt2 = counts.reshape(NTD, NB)
    assert (cnt2.max(axis=0) <= np.array(caps) * P).all(), cnt2.max(axis=0)
    starts = np.concatenate([[0], np.cumsum(counts)[:-1]])
    within = np.arange(len(row_s)) - starts[key_s]

    # slot column within the tile's KT*128 layout (bank-major chunks)
    slot = bank_col_off[key_s % NB] * P + within
    tid = key_s // NB
    idx_pad = np.zeros((NTD, KT * P), np.int16)
    colrel_pad = np.full((NTD, KT * P), -1.0, np.float32)
    idx_pad[tid, slot] = (row_s - (key_s % NB) * BANK).astype(np.int16)
    colrel_pad[tid, slot] = (col_s % DW).astype(np.float32)

    # colrelT: [core][p, t*KT + j]
    cr = colrel_pad.reshape(N_CORES, TPD, KT, P).transpose(0, 3, 1, 2)
    colrelT = np.ascontiguousarray(cr.reshape(N_CORES, P, TPD * KT)).astype(bf16)

    # idx16: per group g, per bank b: flat sequence i over (q, k, p);
    # value at [i % 16, base + i // 16], replicated across partition groups.
    groups = _groups()
    idx_cols_total = sum(tg * cb * 8 for (_, tg) in groups for cb in caps)
    idx16 = np.zeros((N_CORES, 16, idx_cols_total), np.int16)
    idx_pad_c = idx_pad.reshape(N_CORES, TPD, KT * P)
    for c in range(N_CORES):
        base = 0
        for (t0, tg) in groups:
            for b in range(NB):
                cb = caps[b]
                nidx = tg * cb * P
                seq = idx_pad_c[
                    c, t0 : t0 + tg, bank_col_off[b] * P : bank_col_off[b + 1] * P
                ].reshape(nidx)
                idx16[c, :, base : base + nidx // 16] = seq.reshape(
                    nidx // 16, 16
                ).T
                base += nidx // 16
        assert base == idx_cols_total
    idx16 = np.ascontiguousarray(
        np.broadcast_to(
            idx16[:, None, :, :], (N_CORES, 8, 16, idx_cols_total)
        ).reshape(N_CORES, P, idx_cols_total)
    )

    n = x.shape[0]
    xbig = np.zeros((NPAD, D), np.float32)
    xbig[:n] = x
    dbig = np.ones(NPAD, np.float32)
    dbig[:n] = diags
    # diagsT: [p, (bg)*XS_G + k] <-> node (bg)*P*XS_G + p*XS_G + k, matching
    # the contiguous-per-partition xs load layout
    diagsT_h = np.ascontiguousarray(
        dbig.reshape(NT // XS_G, P, XS_G).transpose(1, 0, 2).reshape(P, NT)
    )
    # iotaq: value q at [p, q*KT + c], same for every partition
    iotaq_h = np.ascontiguousarray(
        np.broadcast_to(
            np.repeat(np.arange(DW, dtype=np.float32), KT)[None, :],
            (P, KT * DW),
        )
    ).astype(bf16)
    ident_h = np.eye(DW, dtype=np.float32).astype(bf16)

    scal_h = {
        "m1c": m1, "m2c": m2, "m3c": m3, "e1c": e1, "e2c": e2, "e3c": e3,
    }
    scal_h = {
        k: np.full((DW, 1), np.float32(np.asarray(v).reshape(-1)[0]))
        for k, v in scal_h.items()
    }
    scal_h["e2full"] = np.full((P, 1), np.float32(np.asarray(e2).reshape(-1)[0]))

    in_maps = []
    for k in range(N_CORES):
        lo, hi = k * NPC, (k + 1) * NPC
        in_maps.append(
            {
                "xfull": xbig,
                "diagsT": diagsT_h,
                "ddestT": np.ascontiguousarray(
                    dbig[lo:hi].reshape(TPD, DW).T
                ),
                "xdest": xbig[lo:hi].astype(bf16),
                "idx16": idx16[k],
                "colrelT": colrelT[k],
                "iotaq": iotaq_h,
                "identT": ident_h,
                **scal_h,
            }
        )
    return in_maps


def kernel(x, edge_index, edge_index_id=None, diags=None, m1=None, m2=None,
           m3=None, e1=None, e2=None, e3=None, a=None, **_):
    from concourse.bass_utils import run_bass_kernel_spmd

    x = np.ascontiguousarray(np.asarray(x, dtype=np.float32))
    edge_index = np.asarray(edge_index)
    caps = compute_caps(
        edge_index[1].astype(np.int64), edge_index[0].astype(np.int64)
    )
    in_maps = pack_inputs(
        x, edge_index, np.asarray(diags, dtype=np.float32),
        m1, m2, m3, e1, e2, e3, caps,
    )
    nc = _get_program(caps, N_CORES)
    res = run_bass_kernel_spmd(nc, in_maps, list(range(N_CORES)))
    out = np.concatenate([res.results[k]["out"] for k in range(N_CORES)], axis=0)
    return np.ascontiguousarray(out[:N_NODES])


# revision 5
# speedup vs baseline: 1.2983x; 1.2983x over previous
"""GNN message-passing (NORMADJ graph conv) on 8 Trainium2 NeuronCores. v3.

Math (reference):
    d_e = pow(diags, e)
    gso_1[e]  = m2 * d_e2[row[e]] * d_e3[col[e]]        edge weights
    gso_2[i]  = m1*d_e1[i] + m2*d_e2[i]*d_e3[i] + m3    self-loop weights
    out[i]    = sum_{e: col[e]==i} gso_1[e] * x[row[e]] + gso_2[i] * x[i]

Key identity: d_e3[col] depends only on the destination, so
    out[i] = m2*d_e3[i] * sum_{e: col[e]==i} (d_e2[row[e]] * x[row[e]]) + gso_2[i]*x[i]
i.e. pre-scale x rows once on device (xs = d_e2 * x), scatter-add gathered xs
rows, post-scale per destination node.

Distribution: edges sharded by DESTINATION node range (8 ranges of 12544
nodes); no collectives. Host does index packing only.

v3: destination tiles are 64 nodes wide (DW=64). The scatter one-hot per
chunk is [128 slots, 64 dests] -> the DVE is_equal work halves vs 128-wide
tiles (59M -> 32M elems) at the cost of ~8% more gather padding. PSUM
accumulators are [64, 64]; the self-loop is folded into the accumulation as
an identity matmul over 64 partitions.
"""

import numpy as np
import ml_dtypes

P = 128                       # slots per chunk (PE contraction width)
D = 64
DW = 64                       # destination nodes per tile
N_CORES = 8
N_NODES = 100000
NPC = 12544                   # nodes per core
TPD = NPC // DW               # destination tiles per core (196)
NPAD = N_CORES * NPC          # padded node count (100352)
NT = NPAD // P                # 128-node tiles for the xs phase (784)
NTD = NPAD // DW              # 64-node dest tiles overall (1568)
NB = 4                        # source banks (int16 gather index limit)
BANK = NPAD // NB             # 25088 rows per bank
TPB = NT // NB                # 128-node tiles per bank (196)
T_GROUP = 8                   # dest tiles per dma_gather instruction
XS_G = 14                     # 128-node tiles per xs pre-scale group
GPB = TPB // XS_G             # xs groups per bank (14)
CAPS_DEFAULT = (5, 5, 5, 5)   # chunks per (tile, bank), fitted to the data

_cache = {}


def _groups():
    gs = []
    t = 0
    while t < TPD:
        gs.append((t, min(T_GROUP, TPD - t)))
        t += min(T_GROUP, TPD - t)
    return gs


def blob_layout(caps):
    """Section offsets (int16 units, 128-aligned) for the single input blob."""
    KT = sum(caps)
    groups = _groups()
    idx_cols_total = sum(tg * cb * 8 for (_, tg) in groups for cb in caps)
    sizes = [
        ("xfull", NPAD * D),
        ("diagsT", P * NT * 2),
        ("ddestT", DW * TPD * 2),
        ("xdest", NPC * D),
        ("idx16", P * idx_cols_total),
        ("colrelT", P * TPD * KT),
        ("iotaq", P * KT * DW),
        ("identT", DW * DW),
        ("m1", DW * 2), ("m2", DW * 2), ("m3", DW * 2),
        ("e1", DW * 2), ("e2", DW * 2), ("e3", DW * 2),
        ("e2full", P * 2),
    ]
    offs, o = {}, 0
    for name, n in sizes:
        offs[name] = (o, n)
        o += (n + 127) // 128 * 128
    return offs, o


def _build_program(caps, n_cores, ablate=()):
    import concourse.bacc as bacc
    import concourse.mybir as mybir
    from concourse.tile import TileContext

    f32 = mybir.dt.float32
    bf16 = mybir.dt.bfloat16
    i16 = mybir.dt.int16
    ACT = mybir.ActivationFunctionType

    KT = sum(caps)
    groups = _groups()
    idx_cols_total = sum(tg * cb * 8 for (_, tg) in groups for cb in caps)

    nc = bacc.Bacc(
        "TRN2", target_bir_lowering=False, debug=False, num_devices=n_cores
    )

    offs, total = blob_layout(caps)
    blob = nc.dram_tensor("blob", [1, total], i16, kind="ExternalInput")

    def sect(name, dt, rows, cols):
        o, n = offs[name]
        v = blob[0:1, o : o + n].bitcast(dt)
        return v.rearrange("o (r c) -> (o r) c", c=cols)

    xfull = sect("xfull", bf16, NPAD, D)
    diagsT = sect("diagsT", f32, P, NT)
    ddestT = sect("ddestT", f32, DW, TPD)
    xdest = sect("xdest", bf16, NPC, D)
    idx16 = sect("idx16", i16, P, idx_cols_total)
    colrelT = sect("colrelT", bf16, P, TPD * KT)
    iotaq = sect("iotaq", bf16, P, KT * DW)
    identT = sect("identT", bf16, DW, DW)
    scal_in = {s: sect(s, f32, DW, 1) for s in ("m1", "m2", "m3", "e1", "e2", "e3")}
    scal128 = sect("e2full", f32, P, 1)
    out_d = nc.dram_tensor("out", [NPC, D], f32, kind="ExternalOutput")

    with TileContext(nc) as tc:
        with (
            tc.tile_pool(name="const", bufs=1) as const,
            tc.tile_pool(name="vecs", bufs=1) as vecs,
            tc.tile_pool(name="xsload", bufs=6) as xsload,
            tc.tile_pool(name="xsst", bufs=3) as xsst,
            tc.tile_pool(name="idxp", bufs=5) as idxp,
            tc.tile_pool(name="msgb", bufs=8) as msgbp,
            tc.tile_pool(name="stp", bufs=12) as stp,
            tc.tile_pool(name="outp", bufs=6) as outp,
            tc.tile_pool(name="psum", bufs=8, space="PSUM") as psum,
            tc.tile_pool(name="dram", bufs=1, space="DRAM") as dram,
        ):
            # per-node degree vectors first: the xs-phase critical chain
            # (Ln -> Exp -> muls) hangs off these
            diagsT_sb = vecs.tile([P, NT], f32)
            nc.sync.dma_start(out=diagsT_sb[:], in_=diagsT)
            ddestT_sb = vecs.tile([DW, TPD], f32)
            nc.sync.dma_start(out=ddestT_sb[:], in_=ddestT)

            # resident constants / index tables
            iota_sb = const.tile([P, KT * DW], bf16)
            nc.sync.dma_start(out=iota_sb[:], in_=iotaq)
            colT_sb = const.tile([P, TPD * KT], bf16)
            nc.sync.dma_start(out=colT_sb[:], in_=colrelT)
            ident_sb = const.tile([DW, DW], bf16)
            nc.sync.dma_start(out=ident_sb[:], in_=identT)
            scal = {}
            for s, t in scal_in.items():
                scal[s] = const.tile([DW, 1], f32, name="scal_" + s)
                nc.sync.dma_start(out=scal[s][:], in_=t)
            e2f = const.tile([P, 1], f32, name="e2f")
            nc.sync.dma_start(out=e2f[:], in_=scal128)

            # ---- per-node scalar vectors ----------------------------------
            ln_all = vecs.tile([P, NT], f32)
            nc.scalar.activation(out=ln_all[:], in_=diagsT_sb[:], func=ACT.Ln)
            ln_dest = vecs.tile([DW, TPD], f32)
            nc.scalar.activation(out=ln_dest[:], in_=ddestT_sb[:], func=ACT.Ln)

            d2_all = vecs.tile([P, NT], f32)
            nc.scalar.activation(
                out=d2_all[:], in_=ln_all[:], func=ACT.Exp, scale=e2f[:, 0:1]
            )
            d1d = vecs.tile([DW, TPD], f32)
            nc.scalar.activation(
                out=d1d[:], in_=ln_dest[:], func=ACT.Exp, scale=scal["e1"][:, 0:1]
            )
            d2d = vecs.tile([DW, TPD], f32)
            nc.scalar.activation(
                out=d2d[:], in_=ln_dest[:], func=ACT.Exp, scale=scal["e2"][:, 0:1]
            )
            d3d = vecs.tile([DW, TPD], f32)
            nc.scalar.activation(
                out=d3d[:], in_=ln_dest[:], func=ACT.Exp, scale=scal["e3"][:, 0:1]
            )
            post3 = vecs.tile([DW, TPD], f32)
            nc.vector.tensor_scalar_mul(
                out=post3[:], in0=d3d[:], scalar1=scal["m2"][:, 0:1]
            )
            gso2 = vecs.tile([DW, TPD], f32)
            nc.vector.tensor_mul(out=gso2[:], in0=d2d[:], in1=post3[:])
            t1 = vecs.tile([DW, TPD], f32)
            nc.vector.tensor_scalar_mul(
                out=t1[:], in0=d1d[:], scalar1=scal["m1"][:, 0:1]
            )
            nc.vector.tensor_add(out=gso2[:], in0=gso2[:], in1=t1[:])
            nc.vector.tensor_scalar_add(
                out=gso2[:], in0=gso2[:], scalar1=scal["m3"][:, 0:1]
            )
            # selfw = gso2 / post3: the self-loop rides the shared post3
            # post-scale (requires post3 != 0, i.e. m2 != 0 as in the spec)
            rp3 = vecs.tile([DW, TPD], f32)
            nc.vector.reciprocal(out=rp3[:], in_=post3[:])
            selfw = vecs.tile([DW, TPD], f32)
            nc.vector.tensor_mul(out=selfw[:], in0=gso2[:], in1=rp3[:])

            # prefetch the first gather groups' index tables so the first
            # gathers do not queue behind the xs writes on SP
            prefetched = {}
            _off = 0
            for gi, (t0, tg) in enumerate(groups[:3]):
                gcols = sum(tg * cb * 8 for cb in caps)
                pidx = idxp.tile([P, gcols], i16, name="idx_sb")
                nc.sync.dma_start(out=pidx[:], in_=idx16[:, _off : _off + gcols])
                prefetched[gi] = pidx
                _off += gcols

            # ---- xs = d2[j] * x[j], bf16 packed into the low half of 256B
            # rows (declared f32 so the gather runs in cheap 4-byte mode).
            # one DRAM tensor per bank so bank-b gathers only wait on bank b.
            # within a group, partition p holds XS_G consecutive nodes so the
            # load is one contiguous 3.5KB descriptor per partition.
            xs_banks = [
                dram.tile([BANK, D], f32, name=f"xsb{b}") for b in range(NB)
            ]
            xf_b = xfull.rearrange(
                "(b g p k) d -> b g p k d", b=NB, g=GPB, p=P
            )
            for b in range(NB if "xs" not in ablate else 0):
                xs_v = xs_banks[b][:].rearrange(
                    "(g p k) e -> g p k e", g=GPB, p=P
                )
                for g in range(GPB):
                    xg = xsload.tile([P, XS_G * D], bf16, tag="xg")
                    nc.scalar.dma_start(
                        out=xg[:].rearrange("p (k d) -> p k d", d=D),
                        in_=xf_b[b, g],
                    )
                    xsg = xsst.tile([P, XS_G * D], bf16, tag="xsg")
                    c0 = (b * GPB + g) * XS_G
                    nc.vector.tensor_mul(
                        out=xsg[:].rearrange("p (k d) -> p k d", d=D),
                        in0=xg[:].rearrange("p (k d) -> p k d", d=D),
                        in1=d2_all[:, c0 : c0 + XS_G].to_broadcast([P, XS_G, D]),
                    )
                    nc.sync.dma_start(
                        out=xs_v[g][:, :, 0 : D // 2].bitcast(bf16),
                        in_=xsg[:].rearrange("p (k d) -> p k d", d=D),
                    )

            # ---- main loop: gather / select / matmul-accumulate ------------
            iota3 = iota_sb[:].rearrange("p (q c) -> p q c", c=KT)
            idx_off = 0
            for gi, (t0, tg) in enumerate(groups):
                gcols = sum(tg * cb * 8 for cb in caps)
                if gi in prefetched:
                    idx_sb = prefetched[gi]
                else:
                    idx_sb = idxp.tile([P, gcols], i16, name="idx_sb")
                    nc.sync.dma_start(
                        out=idx_sb[:], in_=idx16[:, idx_off : idx_off + gcols]
                    )
                idx_off += gcols
                msgs = []
                boff = 0
                for b in range(NB):
                    cb = caps[b]
                    nidx = tg * cb * P
                    msgb = msgbp.tile([P, tg * cb * D], f32, tag="msgb")
                    nc.gpsimd.dma_gather(
                        out_ap=msgb[:].rearrange("p (k e) -> p k e", e=D),
                        in_ap=xs_banks[b][:],
                        idxs_ap=idx_sb[:, boff : boff + nidx // 16],
                        num_idxs=nidx,
                        num_idxs_reg=nidx,
                        elem_size=D,
                        single_packet=False,
                    )
                    boff += nidx // 16
                    msgs.append(msgb)

                xdest_g = outp.tile([DW, tg * D], bf16, name="xdest_g")
                nc.sync.dma_start(
                    out=xdest_g[:].rearrange("p (t d) -> p t d", d=D),
                    in_=xdest[t0 * DW : (t0 + tg) * DW, :].rearrange(
                        "(t p) d -> p t d", p=DW
                    ),
                )
                out_g = outp.tile([DW, tg * D], f32, name="out_g")
                for q in range(tg):
                    t = t0 + q
                    sT = (
                        stp.tile([P, KT * DW], bf16, name="sT")
                        if "st" not in ablate
                        else iota_sb
                    )
                    sT3 = sT[:].rearrange("p (q c) -> p q c", c=KT)
                    if "st" not in ablate:
                        nc.vector.tensor_tensor(
                            out=sT3,
                            in0=colT_sb[:, t * KT : (t + 1) * KT]
                            .rearrange("p k -> p () k")
                            .to_broadcast([P, DW, KT]),
                            in1=iota3,
                            op=mybir.AluOpType.is_equal,
                        )
                    # self-loop term, scaled so the shared post3 post-scale
                    # yields gso2 * x[i]
                    self_sb = outp.tile([DW, D], bf16, name="self_sb")
                    nc.vector.tensor_scalar_mul(
                        out=self_sb[:],
                        in0=xdest_g[:, q * D : (q + 1) * D],
                        scalar1=selfw[:, t : t + 1],
                    )
                    acc = psum.tile([DW, D], f32, name="acc")
                    if "mm" in ablate:
                        nc.tensor.matmul(
                            out=acc[:],
                            lhsT=sT3[:, :, 0:1],
                            rhs=msgs[0][
                                :, q * caps[0] * D : q * caps[0] * D + D // 2
                            ].bitcast(bf16),
                            start=True,
                            stop=True,
                        )
                    else:
                        j = 0
                        for b in range(NB):
                            cb = caps[b]
                            for k in range(cb):
                                c0 = (q * cb + k) * D
                                nc.tensor.matmul(
                                    out=acc[:],
                                    lhsT=sT3[:, :, j : j + 1],
                                    rhs=msgs[b][:, c0 : c0 + D // 2].bitcast(bf16),
                                    start=(j == 0),
                                    stop=False,
                                )
                                j += 1
                        nc.tensor.matmul(
                            out=acc[:],
                            lhsT=ident_sb[:],
                            rhs=self_sb[:],
                            start=False,
                            stop=True,
                        )
                    nc.scalar.activation(
                        out=out_g[:, q * D : (q + 1) * D], in_=acc[:],
                        func=ACT.Copy, scale=post3[:, t : t + 1],
                    )
                nc.sync.dma_start(
                    out=out_d[t0 * DW : (t0 + tg) * DW, :].rearrange(
                        "(t p) d -> p t d", p=DW
                    ),
                    in_=out_g[:].rearrange("p (t d) -> p t d", d=D),
                )

    nc.compile()
    return nc


def _get_program(caps, n_cores, ablate=()):
    key = (tuple(caps), n_cores, tuple(ablate))
    if key not in _cache:
        _cache[key] = _build_program(tuple(caps), n_cores, ablate)
    return _cache[key]


def compute_caps(col, row):
    tile = col // DW
    bank = row // BANK
    cnt = np.zeros((NTD, NB), np.int64)
    np.add.at(cnt, (tile, bank), 1)
    caps = np.maximum(np.ceil(cnt.max(axis=0) / P).astype(int), 1)
    return tuple(int(max(c, d)) for c, d in zip(caps, CAPS_DEFAULT))


def pack_inputs(x, edge_index, diags, m1, m2, m3, e1, e2, e3, caps):
    """Host-side index packing. Returns list of per-core input dicts."""
    bf16 = ml_dtypes.bfloat16
    row = np.ascontiguousarray(edge_index[0]).astype(np.int64, copy=False)
    col = np.ascontiguousarray(edge_index[1]).astype(np.int64, copy=False)
    KT = sum(caps)
    bank_col_off = np.concatenate([[0], np.cumsum(caps)])  # chunk offsets

    tile = col // DW
    bank = row // BANK
    key = tile * NB + bank
    order = np.argsort(key, kind="stable")
    row_s = row[order]
    col_s = col[order]
    key_s = key[order]

    counts = np.bincount(key_s, minlength=NTD * NB)
    cnt2 = counts.reshape(NTD, NB)
    assert (cnt2.max(axis=0) <= np.array(caps) * P).all(), cnt2.max(axis=0)
    starts = np.concatenate([[0], np.cumsum(counts)[:-1]])
    within = np.arange(len(row_s)) - starts[key_s]

    # slot column within the tile's KT*128 layout (bank-major chunks)
    slot = bank_col_off[key_s % NB] * P + within
    tid = key_s // NB
    idx_pad = np.zeros((NTD, KT * P), np.int16)
    colrel_pad = np.full((NTD, KT * P), -1.0, np.float32)
    idx_pad[tid, slot] = (row_s - (key_s % NB) * BANK).astype(np.int16)
    colrel_pad[tid, slot] = (col_s % DW).astype(np.float32)

    # colrelT: [core][p, t*KT + j]
    cr = colrel_pad.reshape(N_CORES, TPD, KT, P).transpose(0, 3, 1, 2)
    colrelT = np.ascontiguousarray(cr.reshape(N_CORES, P, TPD * KT)).astype(bf16)

    # idx16: per group g, per bank b: flat sequence i over (q, k, p);
    # value at [i % 16, base + i // 16], replicated across partition groups.
    groups = _groups()
    idx_cols_total = sum(tg * cb * 8 for (_, tg) in groups for cb in caps)
    idx16 = np.zeros((N_CORES, 16, idx_cols_total), np.int16)
    idx_pad_c = idx_pad.reshape(N_CORES, TPD, KT * P)
    for c in range(N_CORES):
        base = 0
        for (t0, tg) in groups:
            for b in range(NB):
                cb = caps[b]
                nidx = tg * cb * P
                seq = idx_pad_c[
                    c, t0 : t0 + tg, bank_col_off[b] * P : bank_col_off[b + 1] * P
                ].reshape(nidx)
                idx16[c, :, base : base + nidx // 16] = seq.reshape(
                    nidx // 16, 16
                ).T
                base += nidx // 16
        assert base == idx_cols_total
    idx16 = np.ascontiguousarray(
        np.broadcast_to(
            idx16[:, None, :, :], (N_CORES, 8, 16, idx_cols_total)
        ).reshape(N_CORES, P, idx_cols_total)
    )

    n = x.shape[0]
    xbig = np.zeros((NPAD, D), np.float32)
    xbig[:n] = x
    xbig_bf = xbig.astype(bf16)
    dbig = np.ones(NPAD, np.float32)
    dbig[:n] = diags
    # diagsT: [p, (bg)*XS_G + k] <-> node (bg)*P*XS_G + p*XS_G + k, matching
    # the contiguous-per-partition xs load layout
    diagsT_h = np.ascontiguousarray(
        dbig.reshape(NT // XS_G, P, XS_G).transpose(1, 0, 2).reshape(P, NT)
    )
    # iotaq: value q at [p, q*KT + c], same for every partition
    iotaq_h = np.ascontiguousarray(
        np.broadcast_to(
            np.repeat(np.arange(DW, dtype=np.float32), KT)[None, :],
            (P, KT * DW),
        )
    ).astype(bf16)
    ident_h = np.eye(DW, dtype=np.float32).astype(bf16)

    scal_h = {
        "m1c": m1, "m2c": m2, "m3c": m3, "e1c": e1, "e2c": e2, "e3c": e3,
    }
    scal_h = {
        k: np.full((DW, 1), np.float32(np.asarray(v).reshape(-1)[0]))
        for k, v in scal_h.items()
    }
    scal_h["e2full"] = np.full((P, 1), np.float32(np.asarray(e2).reshape(-1)[0]))

    offs, total = blob_layout(caps)

    def fill(blob, name, arr):
        o, n = offs[name]
        v = np.ascontiguousarray(arr).view(np.int16).reshape(-1)
        assert v.size == n, (name, v.size, n)
        blob[o : o + n] = v

    in_maps = []
    for k in range(N_CORES):
        lo, hi = k * NPC, (k + 1) * NPC
        blob = np.zeros(total, np.int16)
        fill(blob, "xfull", xbig_bf)
        fill(blob, "diagsT", diagsT_h)
        fill(blob, "ddestT", np.ascontiguousarray(dbig[lo:hi].reshape(TPD, DW).T))
        fill(blob, "xdest", xbig[lo:hi].astype(bf16))
        fill(blob, "idx16", idx16[k])
        fill(blob, "colrelT", colrelT[k])
        fill(blob, "iotaq", iotaq_h)
        fill(blob, "identT", ident_h)
        for s in ("m1", "m2", "m3", "e1", "e2", "e3"):
            fill(blob, s, scal_h[s + "c"])
        fill(blob, "e2full", scal_h["e2full"])
        in_maps.append({"blob": blob[None, :]})
    return in_maps


def kernel(x, edge_index, edge_index_id=None, diags=None, m1=None, m2=None,
           m3=None, e1=None, e2=None, e3=None, a=None, **_):
    from concourse.bass_utils import run_bass_kernel_spmd

    x = np.ascontiguousarray(np.asarray(x, dtype=np.float32))
    edge_index = np.asarray(edge_index)
    caps = compute_caps(
        edge_index[1].astype(np.int64), edge_index[0].astype(np.int64)
    )
    in_maps = pack_inputs(
        x, edge_index, np.asarray(diags, dtype=np.float32),
        m1, m2, m3, e1, e2, e3, caps,
    )
    nc = _get_program(caps, N_CORES)
    res = run_bass_kernel_spmd(nc, in_maps, list(range(N_CORES)))
    out = np.concatenate([res.results[k]["out"] for k in range(N_CORES)], axis=0)
    return np.ascontiguousarray(out[:N_NODES])


# revision 6
# speedup vs baseline: 9.1360x; 7.0368x over previous
"""GNN message-passing (NORMADJ graph conv) on 8 Trainium2 NeuronCores. v3.

Math (reference):
    d_e = pow(diags, e)
    gso_1[e]  = m2 * d_e2[row[e]] * d_e3[col[e]]        edge weights
    gso_2[i]  = m1*d_e1[i] + m2*d_e2[i]*d_e3[i] + m3    self-loop weights
    out[i]    = sum_{e: col[e]==i} gso_1[e] * x[row[e]] + gso_2[i] * x[i]

Key identity: d_e3[col] depends only on the destination, so
    out[i] = m2*d_e3[i] * sum_{e: col[e]==i} (d_e2[row[e]] * x[row[e]]) + gso_2[i]*x[i]
i.e. pre-scale x rows once on device (xs = d_e2 * x), scatter-add gathered xs
rows, post-scale per destination node.

Distribution: edges sharded by DESTINATION node range (8 ranges of 12544
nodes); no collectives. Host does index packing only.

v3: destination tiles are 64 nodes wide (DW=64). The scatter one-hot per
chunk is [128 slots, 64 dests] -> the DVE is_equal work halves vs 128-wide
tiles (59M -> 32M elems) at the cost of ~8% more gather padding. PSUM
accumulators are [64, 64]; the self-loop is folded into the accumulation as
an identity matmul over 64 partitions.
"""

import numpy as np
import ml_dtypes

P = 128                       # slots per chunk (PE contraction width)
D = 64
DW = 112                      # destination nodes per tile
N_CORES = 8
N_NODES = 100000
NPC = 12544                   # nodes per core
TPD = NPC // DW               # destination tiles per core (196)
NPAD = N_CORES * NPC          # padded node count (100352)
NT = NPAD // P                # 128-node tiles for the xs phase (784)
NTD = NPAD // DW              # 64-node dest tiles overall (1568)
NB = 4                        # source banks (int16 gather index limit)
BANK = NPAD // NB             # 25088 rows per bank
TPB = NT // NB                # 128-node tiles per bank (196)
T_GROUP = 4                   # dest tiles per dma_gather instruction
XS_G = 14                     # 128-node tiles per xs pre-scale group
GPB = TPB // XS_G             # xs groups per bank (14)
CAPS_DEFAULT = (8, 8, 8, 8)   # chunks per (tile, bank), fitted to the data

_cache = {}


def _groups():
    gs = []
    t = 0
    while t < TPD:
        gs.append((t, min(T_GROUP, TPD - t)))
        t += min(T_GROUP, TPD - t)
    return gs


def blob_layout(caps):
    """Section offsets (int16 units, 128-aligned) for the single input blob."""
    KT = sum(caps)
    groups = _groups()
    idx_cols_total = sum(tg * cb * 8 for (_, tg) in groups for cb in caps)
    sizes = [
        ("xfull", NPAD * D),
        ("diagsT", P * NT * 2),
        ("ddestT", DW * TPD * 2),
        ("xdest", NPC * D),
        ("idx16", P * idx_cols_total),
        ("colrelT", P * TPD * KT),
        ("iotaq", P * KT * DW),
        ("identT", DW * DW),
        ("m1", DW * 2), ("m2", DW * 2), ("m3", DW * 2),
        ("e1", DW * 2), ("e2", DW * 2), ("e3", DW * 2),
        ("e2full", P * 2),
    ]
    offs, o = {}, 0
    for name, n in sizes:
        offs[name] = (o, n)
        o += (n + 127) // 128 * 128
    return offs, o


def _build_program(caps, n_cores, ablate=()):
    import concourse.bacc as bacc
    import concourse.mybir as mybir
    from concourse.tile import TileContext

    f32 = mybir.dt.float32
    bf16 = mybir.dt.bfloat16
    i16 = mybir.dt.int16
    ACT = mybir.ActivationFunctionType

    KT = sum(caps)
    groups = _groups()
    idx_cols_total = sum(tg * cb * 8 for (_, tg) in groups for cb in caps)

    nq = 4 if "q1" not in ablate else 1
    nc = bacc.Bacc(
        "TRN2", target_bir_lowering=False, debug=False, num_devices=n_cores,
        num_swdge_queues=nq,
    )

    offs, total = blob_layout(caps)
    blob = nc.dram_tensor("blob", [1, total], i16, kind="ExternalInput")

    def sect(name, dt, rows, cols):
        o, n = offs[name]
        v = blob[0:1, o : o + n].bitcast(dt)
        return v.rearrange("o (r c) -> (o r) c", c=cols)

    xfull = sect("xfull", bf16, NPAD, D)
    diagsT = sect("diagsT", f32, P, NT)
    ddestT = sect("ddestT", f32, DW, TPD)
    xdest = sect("xdest", bf16, NPC, D)
    idx16 = sect("idx16", i16, P, idx_cols_total)
    colrelT = sect("colrelT", bf16, P, TPD * KT)
    iotaq = sect("iotaq", bf16, P, KT * DW)
    identT = sect("identT", bf16, DW, DW)
    scal_in = {s: sect(s, f32, DW, 1) for s in ("m1", "m2", "m3", "e1", "e2", "e3")}
    scal128 = sect("e2full", f32, P, 1)
    out_d = nc.dram_tensor("out", [NPC, D], f32, kind="ExternalOutput")

    with TileContext(nc) as tc:
        with (
            tc.tile_pool(name="const", bufs=1) as const,
            tc.tile_pool(name="vecs", bufs=1) as vecs,
            tc.tile_pool(name="xsload", bufs=6) as xsload,
            tc.tile_pool(name="xsst", bufs=3) as xsst,
            tc.tile_pool(name="idxp", bufs=5) as idxp,
            tc.tile_pool(name="msgb", bufs=8) as msgbp,
            tc.tile_pool(name="stp", bufs=6) as stp,
            tc.tile_pool(name="outp", bufs=6) as outp,
            tc.tile_pool(name="psum", bufs=8, space="PSUM") as psum,
            tc.tile_pool(name="dram", bufs=1, space="DRAM") as dram,
        ):
            # per-node degree vectors first: the xs-phase critical chain
            # (Ln -> Exp -> muls) hangs off these
            diagsT_sb = vecs.tile([P, NT], f32)
            nc.sync.dma_start(out=diagsT_sb[:], in_=diagsT)
            ddestT_sb = vecs.tile([DW, TPD], f32)
            nc.sync.dma_start(out=ddestT_sb[:], in_=ddestT)

            # resident constants / index tables
            iota_sb = const.tile([P, KT * DW], bf16)
            nc.sync.dma_start(out=iota_sb[:], in_=iotaq)
            colT_sb = const.tile([P, TPD * KT], bf16)
            nc.sync.dma_start(out=colT_sb[:], in_=colrelT)
            ident_sb = const.tile([DW, DW], bf16)
            nc.sync.dma_start(out=ident_sb[:], in_=identT)
            scal = {}
            for s, t in scal_in.items():
                scal[s] = const.tile([DW, 1], f32, name="scal_" + s)
                nc.sync.dma_start(out=scal[s][:], in_=t)
            e2f = const.tile([P, 1], f32, name="e2f")
            nc.sync.dma_start(out=e2f[:], in_=scal128)

            # ---- per-node scalar vectors ----------------------------------
            ln_all = vecs.tile([P, NT], f32)
            nc.scalar.activation(out=ln_all[:], in_=diagsT_sb[:], func=ACT.Ln)
            ln_dest = vecs.tile([DW, TPD], f32)
            nc.scalar.activation(out=ln_dest[:], in_=ddestT_sb[:], func=ACT.Ln)

            d2_all = vecs.tile([P, NT], f32)
            nc.scalar.activation(
                out=d2_all[:], in_=ln_all[:], func=ACT.Exp, scale=e2f[:, 0:1]
            )
            d1d = vecs.tile([DW, TPD], f32)
            nc.scalar.activation(
                out=d1d[:], in_=ln_dest[:], func=ACT.Exp, scale=scal["e1"][:, 0:1]
            )
            d2d = vecs.tile([DW, TPD], f32)
            nc.scalar.activation(
                out=d2d[:], in_=ln_dest[:], func=ACT.Exp, scale=scal["e2"][:, 0:1]
            )
            d3d = vecs.tile([DW, TPD], f32)
            nc.scalar.activation(
                out=d3d[:], in_=ln_dest[:], func=ACT.Exp, scale=scal["e3"][:, 0:1]
            )
            post3 = vecs.tile([DW, TPD], f32)
            nc.vector.tensor_scalar_mul(
                out=post3[:], in0=d3d[:], scalar1=scal["m2"][:, 0:1]
            )
            gso2 = vecs.tile([DW, TPD], f32)
            nc.vector.tensor_mul(out=gso2[:], in0=d2d[:], in1=post3[:])
            t1 = vecs.tile([DW, TPD], f32)
            nc.vector.tensor_scalar_mul(
                out=t1[:], in0=d1d[:], scalar1=scal["m1"][:, 0:1]
            )
            nc.vector.tensor_add(out=gso2[:], in0=gso2[:], in1=t1[:])
            nc.vector.tensor_scalar_add(
                out=gso2[:], in0=gso2[:], scalar1=scal["m3"][:, 0:1]
            )
            # selfw = gso2 / post3: the self-loop rides the shared post3
            # post-scale (requires post3 != 0, i.e. m2 != 0 as in the spec)
            rp3 = vecs.tile([DW, TPD], f32)
            nc.vector.reciprocal(out=rp3[:], in_=post3[:])
            selfw = vecs.tile([DW, TPD], f32)
            nc.vector.tensor_mul(out=selfw[:], in0=gso2[:], in1=rp3[:])

            # prefetch the first gather groups' index tables so the first
            # gathers do not queue behind the xs writes on SP
            prefetched = {}
            _off = 0
            for gi, (t0, tg) in enumerate(groups[:3]):
                gcols = sum(tg * cb * 8 for cb in caps)
                pidx = idxp.tile([P, gcols], i16, name="idx_sb")
                nc.sync.dma_start(out=pidx[:], in_=idx16[:, _off : _off + gcols])
                prefetched[gi] = pidx
                _off += gcols

            # ---- xs = d2[j] * x[j], bf16 packed into the low half of 256B
            # rows (declared f32 so the gather runs in cheap 4-byte mode).
            # one DRAM tensor per bank so bank-b gathers only wait on bank b.
            # within a group, partition p holds XS_G consecutive nodes so the
            # load is one contiguous 3.5KB descriptor per partition.
            xs_banks = [
                dram.tile([BANK, D], f32, name=f"xsb{b}") for b in range(NB)
            ]
            xf_b = xfull.rearrange(
                "(b g p k) d -> b g p k d", b=NB, g=GPB, p=P
            )
            for b in range(NB if "xs" not in ablate else 0):
                xs_v = xs_banks[b][:].rearrange(
                    "(g p k) e -> g p k e", g=GPB, p=P
                )
                for g in range(GPB):
                    xg = xsload.tile([P, XS_G * D], bf16, tag="xg")
                    nc.scalar.dma_start(
                        out=xg[:].rearrange("p (k d) -> p k d", d=D),
                        in_=xf_b[b, g],
                    )
                    xsg = xsst.tile([P, XS_G * D], bf16, tag="xsg")
                    c0 = (b * GPB + g) * XS_G
                    nc.vector.tensor_mul(
                        out=xsg[:].rearrange("p (k d) -> p k d", d=D),
                        in0=xg[:].rearrange("p (k d) -> p k d", d=D),
                        in1=d2_all[:, c0 : c0 + XS_G].to_broadcast([P, XS_G, D]),
                    )
                    nc.sync.dma_start(
                        out=xs_v[g][:, :, 0 : D // 2].bitcast(bf16),
                        in_=xsg[:].rearrange("p (k d) -> p k d", d=D),
                    )

            # ---- main loop: gather / select / matmul-accumulate ------------
            iota3 = iota_sb[:].rearrange("p (q c) -> p q c", c=KT)
            idx_off = 0
            for gi, (t0, tg) in enumerate(groups):
                gcols = sum(tg * cb * 8 for cb in caps)
                if gi in prefetched:
                    idx_sb = prefetched[gi]
                else:
                    idx_sb = idxp.tile([P, gcols], i16, name="idx_sb")
                    nc.sync.dma_start(
                        out=idx_sb[:], in_=idx16[:, idx_off : idx_off + gcols]
                    )
                idx_off += gcols
                msgs = []
                boff = 0
                for b in range(NB):
                    cb = caps[b]
                    nidx = tg * cb * P
                    msgb = msgbp.tile([P, tg * cb * D], f32, tag="msgb")
                    small = "gs" in ablate
                    nidx_eff = P if small else nidx
                    nc.gpsimd.dma_gather(
                        out_ap=msgb[:, 0 : nidx_eff // P * D].rearrange(
                            "p (k e) -> p k e", e=D
                        ),
                        in_ap=xs_banks[b][:],
                        idxs_ap=idx_sb[:, boff : boff + nidx_eff // 16],
                        num_idxs=nidx_eff,
                        num_idxs_reg=nidx_eff,
                        elem_size=D,
                        single_packet=("sp" in ablate),
                        queue_num=b % nq,
                    )
                    boff += nidx // 16
                    msgs.append(msgb)

                xdest_g = outp.tile([DW, tg * D], bf16, name="xdest_g")
                nc.sync.dma_start(
                    out=xdest_g[:].rearrange("p (t d) -> p t d", d=D),
                    in_=xdest[t0 * DW : (t0 + tg) * DW, :].rearrange(
                        "(t p) d -> p t d", p=DW
                    ),
                )
                out_g = outp.tile([DW, tg * D], f32, name="out_g")
                for q in range(tg):
                    t = t0 + q
                    sT = (
                        stp.tile([P, KT * DW], bf16, name="sT")
                        if "st" not in ablate
                        else iota_sb
                    )
                    sT3 = sT[:].rearrange("p (q c) -> p q c", c=KT)
                    if "st" not in ablate:
                        nc.vector.tensor_tensor(
                            out=sT3,
                            in0=colT_sb[:, t * KT : (t + 1) * KT]
                            .rearrange("p k -> p () k")
                            .to_broadcast([P, DW, KT]),
                            in1=iota3,
                            op=mybir.AluOpType.is_equal,
                        )
                    # self-loop term, scaled so the shared post3 post-scale
                    # yields gso2 * x[i]
                    self_sb = outp.tile([DW, D], bf16, name="self_sb")
                    nc.vector.tensor_scalar_mul(
                        out=self_sb[:],
                        in0=xdest_g[:, q * D : (q + 1) * D],
                        scalar1=selfw[:, t : t + 1],
                    )
                    acc = psum.tile([DW, D], f32, name="acc")
                    if "mm" in ablate:
                        nc.tensor.matmul(
                            out=acc[:],
                            lhsT=sT3[:, :, 0:1],
                            rhs=msgs[0][
                                :, q * caps[0] * D : q * caps[0] * D + D // 2
                            ].bitcast(bf16),
                            start=True,
                            stop=True,
                        )
                    else:
                        j = 0
                        for b in range(NB):
                            cb = caps[b]
                            for k in range(cb):
                                c0 = (q * cb + k) * D
                                nc.tensor.matmul(
                                    out=acc[:],
                                    lhsT=sT3[:, :, j : j + 1],
                                    rhs=msgs[b][:, c0 : c0 + D // 2].bitcast(bf16),
                                    start=(j == 0),
                                    stop=False,
                                )
                                j += 1
                        nc.tensor.matmul(
                            out=acc[:],
                            lhsT=ident_sb[:],
                            rhs=self_sb[:],
                            start=False,
                            stop=True,
                        )
                    nc.scalar.activation(
                        out=out_g[:, q * D : (q + 1) * D], in_=acc[:],
                        func=ACT.Copy, scale=post3[:, t : t + 1],
                    )
                nc.sync.dma_start(
                    out=out_d[t0 * DW : (t0 + tg) * DW, :].rearrange(
                        "(t p) d -> p t d", p=DW
                    ),
                    in_=out_g[:].rearrange("p (t d) -> p t d", d=D),
                )

    nc.compile()
    return nc


def _get_program(caps, n_cores, ablate=()):
    key = (tuple(caps), n_cores, tuple(ablate))
    if key not in _cache:
        _cache[key] = _build_program(tuple(caps), n_cores, ablate)
    return _cache[key]


def compute_caps(col, row):
    tile = col // DW
    bank = row // BANK
    cnt = np.zeros((NTD, NB), np.int64)
    np.add.at(cnt, (tile, bank), 1)
    caps = np.maximum(np.ceil(cnt.max(axis=0) / P).astype(int), 1)
    return tuple(int(max(c, d)) for c, d in zip(caps, CAPS_DEFAULT))


def pack_inputs(x, edge_index, diags, m1, m2, m3, e1, e2, e3, caps):
    """Host-side index packing. Returns list of per-core input dicts."""
    bf16 = ml_dtypes.bfloat16
    row = np.ascontiguousarray(edge_index[0]).astype(np.int64, copy=False)
    col = np.ascontiguousarray(edge_index[1]).astype(np.int64, copy=False)
    KT = sum(caps)
    bank_col_off = np.concatenate([[0], np.cumsum(caps)])  # chunk offsets

    tile = col // DW
    bank = row // BANK
    key = tile * NB + bank
    order = np.argsort(key, kind="stable")
    row_s = row[order]
    col_s = col[order]
    key_s = key[order]

    counts = np.bincount(key_s, minlength=NTD * NB)
    cnt2 = counts.reshape(NTD, NB)
    assert (cnt2.max(axis=0) <= np.array(caps) * P).all(), cnt2.max(axis=0)
    starts = np.concatenate([[0], np.cumsum(counts)[:-1]])
    within = np.arange(len(row_s)) - starts[key_s]

    # slot column within the tile's KT*128 layout (bank-major chunks)
    slot = bank_col_off[key_s % NB] * P + within
    tid = key_s // NB
    idx_pad = np.zeros((NTD, KT * P), np.int16)
    colrel_pad = np.full((NTD, KT * P), -1.0, np.float32)
    idx_pad[tid, slot] = (row_s - (key_s % NB) * BANK).astype(np.int16)
    colrel_pad[tid, slot] = (col_s % DW).astype(np.float32)

    # colrelT: [core][p, t*KT + j]
    cr = colrel_pad.reshape(N_CORES, TPD, KT, P).transpose(0, 3, 1, 2)
    colrelT = np.ascontiguousarray(cr.reshape(N_CORES, P, TPD * KT)).astype(bf16)

    # idx16: per group g, per bank b: flat sequence i over (q, k, p);
    # value at [i % 16, base + i // 16], replicated across partition groups.
    groups = _groups()
    idx_cols_total = sum(tg * cb * 8 for (_, tg) in groups for cb in caps)
    idx16 = np.zeros((N_CORES, 16, idx_cols_total), np.int16)
    idx_pad_c = idx_pad.reshape(N_CORES, TPD, KT * P)
    for c in range(N_CORES):
        base = 0
        for (t0, tg) in groups:
            for b in range(NB):
                cb = caps[b]
                nidx = tg * cb * P
                seq = idx_pad_c[
                    c, t0 : t0 + tg, bank_col_off[b] * P : bank_col_off[b + 1] * P
                ].reshape(nidx)
                idx16[c, :, base : base + nidx // 16] = seq.reshape(
                    nidx // 16, 16
                ).T
                base += nidx // 16
        assert base == idx_cols_total
    idx16 = np.ascontiguousarray(
        np.broadcast_to(
            idx16[:, None, :, :], (N_CORES, 8, 16, idx_cols_total)
        ).reshape(N_CORES, P, idx_cols_total)
    )

    n = x.shape[0]
    xbig = np.zeros((NPAD, D), np.float32)
    xbig[:n] = x
    xbig_bf = xbig.astype(bf16)
    dbig = np.ones(NPAD, np.float32)
    dbig[:n] = diags
    # diagsT: [p, (bg)*XS_G + k] <-> node (bg)*P*XS_G + p*XS_G + k, matching
    # the contiguous-per-partition xs load layout
    diagsT_h = np.ascontiguousarray(
        dbig.reshape(NT // XS_G, P, XS_G).transpose(1, 0, 2).reshape(P, NT)
    )
    # iotaq: value q at [p, q*KT + c], same for every partition
    iotaq_h = np.ascontiguousarray(
        np.broadcast_to(
            np.repeat(np.arange(DW, dtype=np.float32), KT)[None, :],
            (P, KT * DW),
        )
    ).astype(bf16)
    ident_h = np.eye(DW, dtype=np.float32).astype(bf16)

    scal_h = {
        "m1c": m1, "m2c": m2, "m3c": m3, "e1c": e1, "e2c": e2, "e3c": e3,
    }
    scal_h = {
        k: np.full((DW, 1), np.float32(np.asarray(v).reshape(-1)[0]))
        for k, v in scal_h.items()
    }
    scal_h["e2full"] = np.full((P, 1), np.float32(np.asarray(e2).reshape(-1)[0]))

    offs, total = blob_layout(caps)

    def fill(blob, name, arr):
        o, n = offs[name]
        v = np.ascontiguousarray(arr).view(np.int16).reshape(-1)
        assert v.size == n, (name, v.size, n)
        blob[o : o + n] = v

    in_maps = []
    for k in range(N_CORES):
        lo, hi = k * NPC, (k + 1) * NPC
        blob = np.zeros(total, np.int16)
        fill(blob, "xfull", xbig_bf)
        fill(blob, "diagsT", diagsT_h)
        fill(blob, "ddestT", np.ascontiguousarray(dbig[lo:hi].reshape(TPD, DW).T))
        fill(blob, "xdest", xbig[lo:hi].astype(bf16))
        fill(blob, "idx16", idx16[k])
        fill(blob, "colrelT", colrelT[k])
        fill(blob, "iotaq", iotaq_h)
        fill(blob, "identT", ident_h)
        for s in ("m1", "m2", "m3", "e1", "e2", "e3"):
            fill(blob, s, scal_h[s + "c"])
        fill(blob, "e2full", scal_h["e2full"])
        in_maps.append({"blob": blob[None, :]})
    return in_maps


def kernel(x, edge_index, edge_index_id=None, diags=None, m1=None, m2=None,
           m3=None, e1=None, e2=None, e3=None, a=None, **_):
    from concourse.bass_utils import run_bass_kernel_spmd

    x = np.ascontiguousarray(np.asarray(x, dtype=np.float32))
    edge_index = np.asarray(edge_index)
    caps = compute_caps(
        edge_index[1].astype(np.int64), edge_index[0].astype(np.int64)
    )
    in_maps = pack_inputs(
        x, edge_index, np.asarray(diags, dtype=np.float32),
        m1, m2, m3, e1, e2, e3, caps,
    )
    nc = _get_program(caps, N_CORES)
    res = run_bass_kernel_spmd(nc, in_maps, list(range(N_CORES)))
    out = np.concatenate([res.results[k]["out"] for k in range(N_CORES)], axis=0)
    return np.ascontiguousarray(out[:N_NODES])
